# revision 48
# baseline (speedup 1.0000x reference)
"""Trainium2 Bass kernel for nn_Net_LSV: neural local-stochastic-vol Monte Carlo.

Data-parallel over MC paths across 8 NeuronCores (2048 paths/core).
Layout per core: path p = g*128 + i -> partition i, chunk g (i in [0,128), g in [0,16)).

v3 architecture (vs v2; HW-measured 1.152ms vs 1.371ms):
- dB = rho_s*z + c_s*zz precomputed host-side (replaces zz + 2 DVE ops).
- Softplus poly regrouped as (a*x+c)^2 + k: the squared part comes out of
  the Act engine directly (Square with scale/bias); the k*dS contribution
  to cv telescopes to k*(sd-sd0), applied in closed form at the 4 maturity
  events. Saves 4 of the 6 per-step poly ops. vv/pd re-add k cheaply.
- Tamed-Euler denominators 1/(1+y) expanded to 1-y(+y^2) (y<0.05, err<1e-5):
  kills both DVE reciprocals.
- Fold matmuls stream only live maturity columns (idx-sliced).
- x3 via 16 per-chunk pair-transposes into a [2,2048] bf16 psum tile,
  evac'd as two [2,1024] copies (DVE/Act). HW prefers few, wide ops: the
  quarter-width / multi-engine variants measured slower (cross-engine sem
  hops + per-inst overhead dominate; gpsimd/Pool cannot touch PSUM and its
  elementwise ops are Q7-software, ~2.4x slower than modeled).
- bf16 matmul operands; f32 path-state and accumulators.
"""
import numpy as np
from contextlib import ExitStack

import concourse.bass as bass
import concourse.bacc as bacc
import concourse.tile as tile
from concourse import mybir
from concourse.masks import make_identity
from concourse.bass_utils import run_bass_kernel_spmd

F32 = mybir.dt.float32
BF16 = mybir.dt.bfloat16
AF = mybir.ActivationFunctionType
OP = mybir.AluOpType

N_CORES = 8
MC = 16384
P = 128
G = 16
MCC = P * G            # paths per core
NS = 21                # strikes
NM = 4                 # maturities
H1 = 100               # s_vol hidden
VH = 20                # vanilla hedge hidden (x4 maturities = 80)
DV = 20                # v_drift / v_vol hidden (x2 = 40)
HM = NM * VH + 2 * DV  # merged hidden width = 120
CW = NM * NS           # cv width per path-chunk = 84
PSD = F32              # psum dtype for matmul outputs (hw requires fp32)




def _scope(nc, label):
    # record an instruction-counter watermark; the profiler maps each
    # instruction I-N to the label with the largest watermark <= N
    name = nc.get_next_instruction_name()          # consumes one id (gap ok)
    num = int(name.split("-")[-1])
    if not hasattr(nc, "_scope_marks"):
        nc._scope_marks = []
    nc._scope_marks.append((num, label))


def _scope_end(nc):
    pass

def build_program(steps, repeat=1):
    T = len(steps)
    n_ev = sum(1 for s in steps if s["event"] is not None)
    nc = bacc.Bacc()

    # ---------------- DRAM I/O ----------------
    z_d = nc.declare_dram_parameter("z_land", [P, T, G], F32, isOutput=False)
    db_d = nc.declare_dram_parameter("db_land", [P, T, G], F32, isOutput=False)
    w1sv_d = nc.declare_dram_parameter("w1sv_tab", [3, T * H1], BF16, isOutput=False)
    w1m_d = nc.declare_dram_parameter("w1m_tab", [3, T * HM], BF16, isOutput=False)
    w2aug_d = nc.declare_dram_parameter("w2aug", [H1 + 1, H1], BF16, isOutput=False)
    w3aug_d = nc.declare_dram_parameter("w3aug", [H1 + 1, 2], BF16, isOutput=False)
    w2m_d = nc.declare_dram_parameter("w2m", [HM + 2, CW + 2], BF16, isOutput=False)
    krep_d = nc.declare_dram_parameter("krep", [1, max(n_ev, 1) * NS], F32, isOutput=False)
    init_d = nc.declare_dram_parameter("initvals", [1, 4], F32, isOutput=False)
    rhb_d = nc.declare_dram_parameter("rhb", [1, T], F32, isOutput=False)
    out_d = nc.declare_dram_parameter("out", [2 * NM * NS], F32, isOutput=True)

    with tile.TileContext(nc) as tc, ExitStack() as ctx:
        stat = ctx.enter_context(tc.tile_pool(name="stat", bufs=1))
        work = ctx.enter_context(tc.tile_pool(name="work", bufs=2))
        ps_x3 = ctx.enter_context(tc.tile_pool(name="ps_x3", bufs=1, space="PSUM"))
        ps_pdf = ctx.enter_context(tc.tile_pool(name="ps_pdf", bufs=1, space="PSUM"))
        ps_w = ctx.enter_context(tc.tile_pool(name="ps_w", bufs=2, space="PSUM"))

        # ---------- static tiles ----------
        identF = stat.tile([P, P], F32)
        make_identity(nc, identF[:])
        zt = stat.tile([P, T, G], F32)
        nc.sync.dma_start(out=zt[:], in_=z_d[:])
        dbt = stat.tile([P, T, G], F32)
        nc.sync.dma_start(out=dbt[:], in_=db_d[:])
        w1sv_tab = stat.tile([3, T * H1], BF16)
        nc.sync.dma_start(out=w1sv_tab[:], in_=w1sv_d[:])
        w1m_tab = stat.tile([3, T * HM], BF16)
        nc.sync.dma_start(out=w1m_tab[:], in_=w1m_d[:])
        w2aug = stat.tile([H1 + 1, H1], BF16)
        nc.sync.dma_start(out=w2aug[:], in_=w2aug_d[:])
        w3aug = stat.tile([H1 + 1, 2], BF16)
        nc.sync.dma_start(out=w3aug[:], in_=w3aug_d[:])
        w2m = stat.tile([HM + 2, CW + 2], BF16)
        nc.sync.dma_start(out=w2m[:], in_=w2m_d[:])
        krep = stat.tile([P, max(n_ev, 1) * NS], F32)
        nc.sync.dma_start(out=krep[:], in_=krep_d[:].broadcast_to([P, max(n_ev, 1) * NS]))
        initv = stat.tile([P, 4], F32)
        nc.sync.dma_start(out=initv[:], in_=init_d[:].broadcast_to([P, 4]))
        rhb = stat.tile([P, T], F32)
        nc.sync.dma_start(out=rhb[:], in_=rhb_d[:].broadcast_to([P, T]))
        sqb = stat.tile([P, 1], F32)
        nc.gpsimd.memset(sqb[:], float(np.sqrt(0.5)))   # softplus poly inner bias

        ones_col = stat.tile([P, 1], F32)
        nc.gpsimd.memset(ones_col[:], 1.0)

        # ---------- persistent state ----------
        # SV col 2g = slog chunk g, col 2g+1 = v chunk g (interleaved pairs:
        # per-chunk [128,2] pair-transposes write x3p [2,2048] psum directly)
        SV = stat.tile([P, 2 * G], F32)
        SVb = stat.tile([P, 2 * G], BF16)
        sv_s = SV[:, 0:2 * G:2]
        sv_v = SV[:, 1:2 * G:2]
        identB = stat.tile([P, P], BF16)
        make_identity(nc, identB[:])
        sd_a = stat.tile([P, G], F32)
        sd_b = stat.tile([P, G], F32)
        nc.gpsimd.memset(sd_b[:], 0.0)
        x3 = stat.tile([3, MCC], BF16)
        h1s = stat.tile([H1 + 1, MCC], BF16)
        hm = stat.tile([HM + 2, MCC], BF16)
        h2s = stat.tile([H1 + 1, MCC], BF16)
        nc.vector.tensor_copy(x3[0:3, :], ones_col[0:3, :].broadcast_to([3, MCC]))
        # static ones rows (partition base must be a multiple of 32, so fill
        # 96.. ; the step loop overwrites rows 96..H1/HM with activations)
        nc.vector.tensor_copy(h1s[96:H1 + 1, :], ones_col[96:H1 + 1, :].broadcast_to([5, MCC]))
        nc.vector.tensor_copy(h2s[96:H1 + 1, :], ones_col[96:H1 + 1, :].broadcast_to([5, MCC]))
        nc.vector.tensor_copy(hm[96:HM + 2, :], ones_col[96:HM + 2, :].broadcast_to([26, MCC]))

        pd = stat.tile([P, G], F32)
        vd = stat.tile([P, G], F32)
        cv = stat.tile([P, G, CW], F32)
        cvfwd = stat.tile([P, G, CW + 1], BF16)   # col Lw = vv (relative cols)
        outacc = stat.tile([1, 2 * NM * NS], F32)
        nc.gpsimd.memset(outacc[:], 0.0)

        sd_tiles = [sd_a, sd_b]

        for rep in range(repeat):
          # per-repeat state init
          nc.vector.tensor_copy(sv_s, initv[:, 0:1].broadcast_to([P, G]))
          nc.vector.tensor_copy(sv_v, initv[:, 1:2].broadcast_to([P, G]))
          nc.vector.tensor_copy(sd_a[:], initv[:, 2:3].broadcast_to([P, G]))
          nc.gpsimd.memset(cv[:], 0.0)
          pending_cv = None                     # deferred cv += cvfwd*dS emitter

          for t, st in enumerate(steps):
            t0, h, sqh = st["t0"], st["h"], st["sqh"]
            rate = st["rate"]
            idx = st["idx"]
            Lw = CW - idx * NS                # live cv width
            spw = Lw + 2                      # live + vv + vd fold cols
            sd_old = sd_tiles[t % 2]
            sd_new = sd_tiles[(t + 1) % 2]

            _scope(nc, "x3")
            # ---- x3: bf16 cast + 16 pair-transposes into [2,2048] psum,
            #      evac'd progressively as four [2,512] copies ----
            nc.vector.tensor_copy(SVb[:], SV[:])
            x3p = ps_x3.tile([2, MCC], BF16, tag="x3p")
            for g in range(G):
                nc.tensor.transpose(x3p[0:2, g * P:(g + 1) * P],
                                    SVb[:, 2 * g:2 * g + 2], identB[:])
            nc.vector.tensor_copy(x3[0:2, 0:1024], x3p[0:2, 0:1024])
            nc.scalar.copy(x3[0:2, 1024:2048], x3p[0:2, 1024:2048])

            # deferred cv-update of the previous step: its deps are long done,
            # so it fills engine idle slots during this step's L1/L2 phase
            if pending_cv is not None:
                pending_cv()
                pending_cv = None

            _scope(nc, "L1")
            # ---- L1 matmuls: sv (100 wide) and merged vh+vdvv (120 wide) ----
            # evacs quarter-width so consumers gate on 512-col granularity
            w1sv_t = w1sv_tab[:, t * H1:(t + 1) * H1]
            w1m_t = w1m_tab[:, t * HM:(t + 1) * HM]
            l1sv = []
            for half in range(2):
                ps = ps_w.tile([P, 8, P], PSD, tag="w")
                for q2 in range(2):
                    nc.tensor.matmul(ps[0:H1].rearrange("p a b -> p (a b)")[:, q2 * 512:(q2 + 1) * 512],
                                     w1sv_t, x3[:, half * 1024 + q2 * 512: half * 1024 + (q2 + 1) * 512])
                l1sv.append(ps)
            nc.scalar.activation(h1s[0:H1, 0:1024], l1sv[0][0:H1].rearrange("p a b -> p (a b)"),
                                 AF.Relu, bias=0.0, scale=1.0)
            nc.vector.tensor_scalar(h1s[0:H1, 1024:2048], l1sv[1][0:H1].rearrange("p a b -> p (a b)"),
                                    0.0, None, OP.max)
            l1m = []
            for half in range(2):
                ps = ps_w.tile([P, 8, P], PSD, tag="w")
                for q2 in range(2):
                    nc.tensor.matmul(ps[0:HM].rearrange("p a b -> p (a b)")[:, q2 * 512:(q2 + 1) * 512],
                                     w1m_t, x3[:, half * 1024 + q2 * 512: half * 1024 + (q2 + 1) * 512])
                l1m.append(ps)
            nc.scalar.activation(hm[0:HM, 0:1024], l1m[0][0:HM].rearrange("p a b -> p (a b)"),
                                 AF.Relu, bias=0.0, scale=1.0)
            nc.vector.tensor_scalar(hm[0:HM, 1024:2048], l1m[1][0:HM].rearrange("p a b -> p (a b)"),
                                    0.0, None, OP.max)

            _scope(nc, "L2")
            # ---- sv L2 ----
            l2 = []
            for half in range(2):
                ps = ps_w.tile([P, 8, P], PSD, tag="w")
                for q in range(2):
                    nc.tensor.matmul(ps[0:H1].rearrange("p a b -> p (a b)")[:, q * 512:(q + 1) * 512],
                                     w2aug[:], h1s[:, half * 1024 + q * 512: half * 1024 + (q + 1) * 512])
                l2.append(ps)
            nc.scalar.activation(h2s[0:H1, 0:1024], l2[0][0:H1].rearrange("p a b -> p (a b)"),
                                 AF.Relu, bias=0.0, scale=1.0)
            nc.vector.tensor_scalar(h2s[0:H1, 1024:2048], l2[1][0:H1].rearrange("p a b -> p (a b)"),
                                    0.0, None, OP.max)

            _scope(nc, "fold")
            # ---- merged fold: live cv cols + vv + vd ----
            # softplus(x) = (SQA*x + SQB)^2 + SPK exactly (= ln2 + x/2 + x^2/8);
            # cvfwd holds only the squared part -- the SPK*dS cv contribution
            # telescopes to SPK*(sd-sd0), applied at maturity events.
            SQA = float(1.0 / np.sqrt(8.0))
            SPK = float(np.log(2.0) - 0.5)
            w2m_live = w2m[:, idx * NS:CW + 2]
            for half in range(2):
                hs = slice(half * 8, (half + 1) * 8)
                ps = ps_w.tile([P, 8, P], PSD, tag="w")
                for gl in range(8):
                    g = half * 8 + gl
                    nc.tensor.matmul(ps[:, gl, 0:spw], hm[:, g * P:(g + 1) * P], w2m_live)
                nc.scalar.activation(cvfwd[:, hs, 0:Lw + 1], ps[:, :, 0:Lw + 1],
                                     AF.Square, bias=sqb[:, 0:1], scale=SQA)
                nc.vector.tensor_copy(vd[:, hs], ps[:, :, Lw + 1])
            vv2_ap = cvfwd[:, :, Lw]          # squared part; true vv = vv2 + SPK

            _scope(nc, "pdfold")
            # ---- sv L3 fold -> pd = (SQA*x+SQB)^2 + SPK ----
            pdf = ps_pdf.tile([P, 2 * G], PSD, tag="pdf")
            for g in range(G):
                nc.tensor.matmul(pdf[:, 2 * g:2 * g + 2], h2s[:, g * P:(g + 1) * P], w3aug[:])
            x2k = work.tile([P, G], F32, tag="x2k")
            nc.scalar.activation(x2k[:], pdf[:, 0:2 * G:2], AF.Square, bias=sqb[:, 0:1], scale=SQA)
            nc.vector.tensor_scalar(pd[:], x2k[:], SPK, None, OP.add)

            _scope(nc, "V")
            # ---- V update: V += vd*h + (vv2+SPK)*dB ----
            dB_t = dbt[:, t, :]
            z_t = zt[:, t, :]
            vv = work.tile([P, G], F32, tag="vv")
            nc.vector.tensor_scalar(vv[:], vv2_ap, SPK, None, OP.add)
            vtmp = work.tile([P, G], F32, tag="vtmp")
            nc.vector.scalar_tensor_tensor(vtmp[:], vd[:], float(h), sv_v, OP.mult, OP.add)
            vvdB = work.tile([P, G], F32, tag="vvdB")
            nc.vector.tensor_tensor(vvdB[:], vv[:], dB_t, OP.mult)
            nc.vector.tensor_tensor(sv_v, vtmp[:], vvdB[:], OP.add)

            _scope(nc, "slog")
            # ---- Slog update ----
            # u = drift*h*(1-|drift|*sqh) + pd*z*sqh*(1 - pd*sqh + (pd*sqh)^2)
            # (1/(1+y) ~ 1-y(+y^2); y<0.05 so err < 1e-5 relative)
            pd2 = work.tile([P, G], F32, tag="pd2")
            nc.vector.tensor_tensor(pd2[:], pd[:], pd[:], OP.mult)
            drift = work.tile([P, G], F32, tag="drift")
            nc.vector.tensor_scalar(drift[:], pd2[:], -0.5, float(rate), OP.mult, OP.add)
            absd = work.tile([P, G], F32, tag="absd")
            nc.scalar.activation(absd[:], drift[:], AF.Abs, bias=0.0, scale=1.0)
            w1t = work.tile([P, G], F32, tag="w1t")
            nc.vector.tensor_scalar(w1t[:], absd[:], float(-h * sqh), float(h), OP.mult, OP.add)
            term1 = work.tile([P, G], F32, tag="term1")
            nc.vector.tensor_tensor(term1[:], drift[:], w1t[:], OP.mult)
            a2 = work.tile([P, G], F32, tag="a2")
            nc.vector.tensor_scalar(a2[:], pd[:], float(-sqh), 1.0, OP.mult, OP.add)
            w2t = work.tile([P, G], F32, tag="w2t")
            nc.vector.scalar_tensor_tensor(w2t[:], pd2[:], float(sqh * sqh), a2[:], OP.mult, OP.add)
            pdz = work.tile([P, G], F32, tag="pdz")
            nc.vector.tensor_tensor(pdz[:], pd[:], z_t, OP.mult)
            term2 = work.tile([P, G], F32, tag="term2")
            nc.vector.scalar_tensor_tensor(term2[:], pdz[:], float(sqh), w2t[:], OP.mult, OP.mult)
            u = work.tile([P, G], F32, tag="u")
            nc.vector.tensor_tensor(u[:], term1[:], term2[:], OP.add)
            nc.vector.tensor_tensor(sv_s, sv_s, u[:], OP.add)

            _scope(nc, "exp")
            # ---- sd_new = sd_old * exp(u - r*h) ----
            equ = work.tile([P, G], F32, tag="equ")
            nc.scalar.activation(equ[:], u[:], AF.Exp, bias=rhb[:, t:t + 1], scale=1.0)
            nc.vector.tensor_tensor(sd_new[:], sd_old[:], equ[:], OP.mult)
            dS = work.tile([P, G], F32, tag="dS")
            nc.vector.tensor_tensor(dS[:], sd_new[:], sd_old[:], OP.subtract)

            # ---- cv += cvfwd * dS (live cols; chunks split DVE/Pool) ----
            # Emitted deferred (top of next step) unless this step has an
            # event, so the next step's x3/L1 head work leads the queues.
            def emit_cv(dS=dS, idx=idx, Lw=Lw, tno=t):
                _scope(nc, "cv")
                cvds = work.tile([P, G, CW], F32, tag="cvds")
                h0 = slice(0, 8)
                h1_ = slice(8, 16)
                dS_b0 = dS[:, h0].unsqueeze(-1).broadcast_to([P, 8, Lw])
                dS_b1 = dS[:, h1_].unsqueeze(-1).broadcast_to([P, 8, Lw])
                nc.vector.tensor_tensor(cvds[:, h0, 0:Lw], cvfwd[:, h0, 0:Lw], dS_b0, OP.mult)
                nc.vector.tensor_tensor(cvds[:, h1_, 0:Lw], cvfwd[:, h1_, 0:Lw], dS_b1, OP.mult)
                nc.vector.tensor_tensor(cv[:, h0, idx * NS:CW], cv[:, h0, idx * NS:CW],
                                        cvds[:, h0, 0:Lw], OP.add)
                nc.vector.tensor_tensor(cv[:, h1_, idx * NS:CW], cv[:, h1_, idx * NS:CW],
                                        cvds[:, h1_, 0:Lw], OP.add)

            if st["event"] is not None:
                emit_cv()
            else:
                pending_cv = emit_cv

            _scope(nc, "event")
            # ---- maturity event ----
            if st["event"] is not None:
                ev, kslots = st["event"]
                sd0v = st["sd0"]
                pay = work.tile([P, G, NS], F32, tag="pay")
                sd_bc = sd_new[:].unsqueeze(-1).broadcast_to([P, G, NS])
                kd_bc = krep[:, ev * NS:(ev + 1) * NS].unsqueeze(1).broadcast_to([P, G, NS])
                nc.vector.tensor_tensor(pay[:], sd_bc, kd_bc, OP.subtract)
                nc.vector.tensor_scalar(pay[:], pay[:], 0.0, None, OP.max)
                # corr = SPK*(sd_new - sd0): the deferred softplus constant
                corr = work.tile([P, G], F32, tag="corr")
                nc.vector.tensor_scalar(corr[:], sd_new[:], SPK, float(-SPK * sd0v),
                                        OP.mult, OP.add)
                price = work.tile([P, G, NS], F32, tag="price")
                nc.vector.tensor_tensor(price[:], pay[:],
                                        corr[:].unsqueeze(-1).broadcast_to([P, G, NS]), OP.subtract)
                nc.vector.tensor_tensor(price[:], price[:],
                                        cv[:, :, idx * NS:(idx + 1) * NS], OP.subtract)
                price2 = work.tile([P, G, NS], F32, tag="price2")
                nc.vector.tensor_tensor(price2[:], price[:], price[:], OP.mult)
                red = work.tile([P, 2 * NS], F32, tag="red")
                nc.vector.tensor_reduce(red[:, 0:NS], price[:].transpose([0, 2, 1]),
                                        mybir.AxisListType.X, OP.add)
                nc.vector.tensor_reduce(red[:, NS:2 * NS], price2[:].transpose([0, 2, 1]),
                                        mybir.AxisListType.X, OP.add)
                pred = ps_pdf.tile([1, 2 * NS], F32, tag="pred")
                nc.tensor.matmul(pred[:], ones_col[:], red[:])
                for k in kslots:
                    nc.scalar.copy(outacc[0:1, k * NS:(k + 1) * NS], pred[0:1, 0:NS])
                    nc.scalar.copy(outacc[0:1, NM * NS + k * NS:NM * NS + (k + 1) * NS],
                                   pred[0:1, NS:2 * NS])

          if pending_cv is not None:
              pending_cv()
              pending_cv = None

        _scope_end(nc)
        nc.sync.dma_start(out=out_d[:].unsqueeze(0), in_=outacc[:])

    nc.compile()
    return nc


def _prep(inputs):
    """Host-side preprocessing -> (steps, arrays-for-in_maps, shards, written, T)."""
    import ml_dtypes
    bf = ml_dtypes.bfloat16
    f = lambda k: np.asarray(inputs[k], dtype=np.float32)
    S0 = float(f("S0")); rate = float(f("rate"))
    z = f("z"); zz = f("zz")
    timegrid = f("timegrid"); strikes = f("strikes")
    v0 = float(f("v0")[0]); rho = float(f("rho")[0])
    mats = np.asarray(inputs["maturities"]).astype(np.int64)

    rho_t = float(np.tanh(np.float32(rho)))
    c_t = float(np.sqrt(np.float32(1.0) - np.float32(rho_t) ** 2))
    V0 = float(1.0 / (1.0 + np.exp(-np.float32(v0))) * 0.5)
    slog0 = float(np.log(np.float32(S0)))

    days = np.round(timegrid * 365.0).astype(np.int64)
    le = days[1:, None] <= mats[None, :]
    idx_net = np.argmax(le, axis=1)
    is_mat = np.any(days[1:, None] == mats[None, :], axis=1)
    if not is_mat.any():
        return None

    T = int(np.max(np.nonzero(is_mat)[0])) + 1
    steps = []
    krep_list = []
    ev = 0
    for t in range(T):
        t0 = float(timegrid[t]); t1 = float(timegrid[t + 1])
        h = float(np.float32(t1) - np.float32(t0))
        sqh = float(np.sqrt(np.float32(h)))
        event = None
        if is_mat[t]:
            k = int(idx_net[t])
            event = (ev, [k])
            krep_list.append(np.exp(-rate * t1).astype(np.float32) * strikes)
            ev += 1
        steps.append(dict(
            t0=t0, h=h, sqh=sqh, rho_s=rho_t * sqh, c_s=c_t * sqh, rate=rate,
            sd0=float(np.exp(np.float32(slog0) - np.float32(rate) * timegrid[0])),
            idx=int(idx_net[t]), event=event,
        ))

    # weight repacks
    sv_W1 = f("sv_W1"); sv_b1 = f("sv_b1"); sv_W2 = f("sv_W2"); sv_b2 = f("sv_b2")
    sv_W3 = f("sv_W3"); sv_b3 = f("sv_b3")
    vh_W1 = f("vh_W1"); vh_b1 = f("vh_b1"); vh_W2 = f("vh_W2"); vh_b2 = f("vh_b2")
    vd_W1 = f("vd_W1"); vd_b1 = f("vd_b1"); vd_W2 = f("vd_W2"); vd_b2 = f("vd_b2")
    vv_W1 = f("vv_W1"); vv_b1 = f("vv_b1"); vv_W2 = f("vv_W2"); vv_b2 = f("vv_b2")
    t0s = timegrid[:T].astype(np.float32)

    arrs = {}
    # w1sv_tab rows (slog, v, bias(t))
    w1sv_3 = np.zeros((3, T * H1), np.float32)
    for t in range(T):
        w1sv_3[0, t * H1:(t + 1) * H1] = sv_W1[1]
        w1sv_3[1, t * H1:(t + 1) * H1] = sv_W1[2]
        w1sv_3[2, t * H1:(t + 1) * H1] = sv_b1 + sv_W1[0] * t0s[t]
    arrs["w1sv_tab"] = w1sv_3
    # w1m_tab: cols 0-79 vh (slog row + t-bias), cols 80-119 vdvv (v row + bias)
    w1m_3 = np.zeros((3, T * HM), np.float32)
    vh_w1_t = vh_W1[:, 0, :].reshape(NM * VH)   # t0 weight
    vh_w1_s = vh_W1[:, 1, :].reshape(NM * VH)   # slog weight
    vh_b1f = vh_b1.reshape(NM * VH)
    for t in range(T):
        c0 = t * HM
        w1m_3[0, c0:c0 + NM * VH] = vh_w1_s
        w1m_3[2, c0:c0 + NM * VH] = vh_b1f + vh_w1_t * t0s[t]
        w1m_3[1, c0 + NM * VH:c0 + NM * VH + DV] = vd_W1[0]
        w1m_3[2, c0 + NM * VH:c0 + NM * VH + DV] = vd_b1
        w1m_3[1, c0 + NM * VH + DV:c0 + HM] = vv_W1[0]
        w1m_3[2, c0 + NM * VH + DV:c0 + HM] = vv_b1
    arrs["w1m_tab"] = w1m_3
    arrs["w2aug"] = np.concatenate([sv_W2, sv_b2[None, :]], 0)
    arrs["w3aug"] = np.concatenate(
        [np.concatenate([sv_W3, sv_b3[None, :]], 0), np.zeros((H1 + 1, 1), np.float32)], 1)
    # w2m [122, 86]: vh block-diag + vv/vd columns; rows 120/121 are the two ones rows
    w2m = np.zeros((HM + 2, CW + 2), np.float32)
    for k in range(NM):
        w2m[k * VH:(k + 1) * VH, k * NS:(k + 1) * NS] = vh_W2[k]
        w2m[HM, k * NS:(k + 1) * NS] = vh_b2[k]
    w2m[NM * VH + DV:HM, CW] = vv_W2[:, 0]
    w2m[NM * VH:NM * VH + DV, CW + 1] = vd_W2[:, 0]
    w2m[HM + 1, CW] = vv_b2[0]
    w2m[HM + 1, CW + 1] = vd_b2[0]
    arrs["w2m"] = w2m
    for k in ("w1sv_tab", "w1m_tab", "w2aug", "w3aug", "w2m"):
        arrs[k] = np.ascontiguousarray(arrs[k]).astype(bf)

    if krep_list:
        arrs["krep"] = np.concatenate(krep_list)[None, :].astype(np.float32)
    else:
        arrs["krep"] = np.zeros((1, NS), np.float32)
    sd0 = float(np.exp(np.float32(slog0) - np.float32(rate) * timegrid[0]))
    arrs["initvals"] = np.array([[slog0, V0, sd0, 1.0]], np.float32)
    arrs["rhb"] = np.array([[-s["rate"] * s["h"] for s in steps]], np.float32)

    # z / dB shards: [MCC, T] slice -> [G, P, T] -> [P, T, G]
    rho_s = np.array([s["rho_s"] for s in steps], np.float32)  # [T]
    c_s = np.array([s["c_s"] for s in steps], np.float32)
    zshards, dbshards = [], []
    for c in range(N_CORES):
        zc = z[c * MCC:(c + 1) * MCC, :T]
        zzc = zz[c * MCC:(c + 1) * MCC, :T]
        dbc = rho_s[None, :] * zc + c_s[None, :] * zzc
        for src, lst in ((zc, zshards), (dbc, dbshards)):
            s = src.reshape(G, P, T).transpose(1, 2, 0)
            lst.append(np.ascontiguousarray(s, dtype=np.float32))

    written = sorted({k for s in steps if s["event"] for k in s["event"][1]})
    return steps, arrs, zshards, dbshards, written, T


_CACHE = {}


def kernel(**inputs) -> np.ndarray:
    prep = _prep(inputs)
    if prep is None:
        return np.zeros((2, NM, NS), np.float32)
    steps, arrs, zshards, dbshards, written, T = prep

    key = (T,) + tuple(
        (s["t0"], s["h"], s["rho_s"], s["c_s"], s["rate"], s["idx"],
         None if s["event"] is None else (s["event"][0], tuple(s["event"][1])))
        for s in steps)
    nc = _CACHE.get(key)
    if nc is None:
        nc = build_program(steps)
        _CACHE[key] = nc

    in_maps = []
    for c in range(N_CORES):
        m = dict(arrs)
        m["z_land"] = zshards[c]
        m["db_land"] = dbshards[c]
        in_maps.append(m)

    res = run_bass_kernel_spmd(nc, in_maps, list(range(N_CORES)))
    sums = np.zeros(2 * NM * NS, np.float64)
    for c in range(N_CORES):
        sums += res.results[c]["out"].astype(np.float64)
    s1 = sums[:NM * NS].reshape(NM, NS)
    s2 = sums[NM * NS:].reshape(NM, NS)
    pv = np.zeros((NM, NS), np.float64)
    pvar = np.zeros((NM, NS), np.float64)
    for k in written:
        pv[k] = s1[k] / MC
        pvar[k] = (s2[k] - MC * pv[k] ** 2) / (MC - 1)
    return np.stack([pv, pvar]).astype(np.float32)


# revision 52
# speedup vs baseline: 1.1066x; 1.1066x over previous
"""Trainium2 Bass kernel for nn_Net_LSV: neural local-stochastic-vol Monte Carlo.

Data-parallel over MC paths across 8 NeuronCores (2048 paths/core).
Layout per core: path p = g*128 + i -> partition i, chunk g (i in [0,128), g in [0,16)).

v3 architecture (vs v2; HW-measured 1.152ms vs 1.371ms):
- dB = rho_s*z + c_s*zz precomputed host-side (replaces zz + 2 DVE ops).
- Softplus poly regrouped as (a*x+c)^2 + k: the squared part comes out of
  the Act engine directly (Square with scale/bias); the k*dS contribution
  to cv telescopes to k*(sd-sd0), applied in closed form at the 4 maturity
  events. Saves 4 of the 6 per-step poly ops. vv/pd re-add k cheaply.
- Tamed-Euler denominators 1/(1+y) expanded to 1-y(+y^2) (y<0.05, err<1e-5):
  kills both DVE reciprocals.
- Fold matmuls stream only live maturity columns (idx-sliced).
- x3 via 16 per-chunk pair-transposes into a [2,2048] bf16 psum tile,
  evac'd as two [2,1024] copies (DVE/Act). HW prefers few, wide ops: the
  quarter-width / multi-engine variants measured slower (cross-engine sem
  hops + per-inst overhead dominate; gpsimd/Pool cannot touch PSUM and its
  elementwise ops are Q7-software, ~2.4x slower than modeled).
- bf16 matmul operands; f32 path-state and accumulators.
"""
import numpy as np
from contextlib import ExitStack

import concourse.bass as bass
import concourse.bacc as bacc
import concourse.tile as tile
from concourse import mybir
from concourse.masks import make_identity
from concourse.bass_utils import run_bass_kernel_spmd

F32 = mybir.dt.float32
BF16 = mybir.dt.bfloat16
AF = mybir.ActivationFunctionType
OP = mybir.AluOpType

N_CORES = 8
MC = 16384
P = 128
G = 16
MCC = P * G            # paths per core
NS = 21                # strikes
NM = 4                 # maturities
H1 = 100               # s_vol hidden
VH = 20                # vanilla hedge hidden (x4 maturities = 80)
DV = 20                # v_drift / v_vol hidden (x2 = 40)
HM = NM * VH + 2 * DV  # merged hidden width = 120
CW = NM * NS           # cv width per path-chunk = 84
PSD = F32              # psum dtype for matmul outputs (hw requires fp32)




def _scope(nc, label):
    # record an instruction-counter watermark; the profiler maps each
    # instruction I-N to the label with the largest watermark <= N
    name = nc.get_next_instruction_name()          # consumes one id (gap ok)
    num = int(name.split("-")[-1])
    if not hasattr(nc, "_scope_marks"):
        nc._scope_marks = []
    nc._scope_marks.append((num, label))


def _scope_end(nc):
    pass

def build_program(steps, repeat=1):
    T = len(steps)
    n_ev = sum(1 for s in steps if s["event"] is not None)
    nc = bacc.Bacc()

    # ---------------- DRAM I/O ----------------
    z_d = nc.declare_dram_parameter("z_land", [P, T, G], F32, isOutput=False)
    db_d = nc.declare_dram_parameter("db_land", [P, T, G], F32, isOutput=False)
    w1sv_d = nc.declare_dram_parameter("w1sv_tab", [3, T * H1], BF16, isOutput=False)
    w1m_d = nc.declare_dram_parameter("w1m_tab", [3, T * HM], BF16, isOutput=False)
    w2aug_d = nc.declare_dram_parameter("w2aug", [H1 + 1, H1], BF16, isOutput=False)
    w3aug_d = nc.declare_dram_parameter("w3aug", [H1 + 1, 2], BF16, isOutput=False)
    w2m_d = nc.declare_dram_parameter("w2m", [HM + 2, CW + 2], BF16, isOutput=False)
    krep_d = nc.declare_dram_parameter("krep", [1, max(n_ev, 1) * NS], F32, isOutput=False)
    init_d = nc.declare_dram_parameter("initvals", [1, 4], F32, isOutput=False)
    rhb_d = nc.declare_dram_parameter("rhb", [1, T], F32, isOutput=False)
    out_d = nc.declare_dram_parameter("out", [2 * NM * NS], F32, isOutput=True)

    with tile.TileContext(nc) as tc, ExitStack() as ctx:
        stat = ctx.enter_context(tc.tile_pool(name="stat", bufs=1))
        work = ctx.enter_context(tc.tile_pool(name="work", bufs=2))
        ps_x3 = ctx.enter_context(tc.tile_pool(name="ps_x3", bufs=1, space="PSUM"))
        ps_pdf = ctx.enter_context(tc.tile_pool(name="ps_pdf", bufs=1, space="PSUM"))
        ps_w = ctx.enter_context(tc.tile_pool(name="ps_w", bufs=2, space="PSUM"))

        # ---------- static tiles ----------
        identF = stat.tile([P, P], F32)
        make_identity(nc, identF[:])
        zt = stat.tile([P, T, G], F32)
        nc.sync.dma_start(out=zt[:], in_=z_d[:])
        dbt = stat.tile([P, T, G], F32)
        nc.sync.dma_start(out=dbt[:], in_=db_d[:])
        w1sv_tab = stat.tile([3, T * H1], BF16)
        nc.sync.dma_start(out=w1sv_tab[:], in_=w1sv_d[:])
        w1m_tab = stat.tile([3, T * HM], BF16)
        nc.sync.dma_start(out=w1m_tab[:], in_=w1m_d[:])
        w2aug = stat.tile([H1 + 1, H1], BF16)
        nc.sync.dma_start(out=w2aug[:], in_=w2aug_d[:])
        w3aug = stat.tile([H1 + 1, 2], BF16)
        nc.sync.dma_start(out=w3aug[:], in_=w3aug_d[:])
        w2m = stat.tile([HM + 2, CW + 2], BF16)
        nc.sync.dma_start(out=w2m[:], in_=w2m_d[:])
        krep = stat.tile([P, max(n_ev, 1) * NS], F32)
        nc.sync.dma_start(out=krep[:], in_=krep_d[:].broadcast_to([P, max(n_ev, 1) * NS]))
        initv = stat.tile([P, 4], F32)
        nc.sync.dma_start(out=initv[:], in_=init_d[:].broadcast_to([P, 4]))
        rhb = stat.tile([P, T], F32)
        nc.sync.dma_start(out=rhb[:], in_=rhb_d[:].broadcast_to([P, T]))
        sqb = stat.tile([P, 1], F32)
        nc.gpsimd.memset(sqb[:], float(np.sqrt(0.5)))   # softplus poly inner bias

        ones_col = stat.tile([P, 1], F32)
        nc.gpsimd.memset(ones_col[:], 1.0)

        # ---------- persistent state ----------
        # SV col 2g = slog chunk g, col 2g+1 = v chunk g (interleaved pairs:
        # per-chunk [128,2] pair-transposes write x3p [2,2048] psum directly)
        SV = stat.tile([P, 2 * G], F32)
        SVb = stat.tile([P, 2 * G], BF16)
        sv_s = SV[:, 0:2 * G:2]
        sv_v = SV[:, 1:2 * G:2]
        identB = stat.tile([P, P], BF16)
        make_identity(nc, identB[:])
        sd_a = stat.tile([P, G], F32)
        sd_b = stat.tile([P, G], F32)
        nc.gpsimd.memset(sd_b[:], 0.0)
        x3 = stat.tile([3, MCC], BF16)
        h1s = stat.tile([H1 + 1, MCC], BF16)
        hm = stat.tile([HM + 2, MCC], BF16)
        h2s = stat.tile([H1 + 1, MCC], BF16)
        nc.vector.tensor_copy(x3[0:3, :], ones_col[0:3, :].broadcast_to([3, MCC]))
        # static ones rows (partition base must be a multiple of 32, so fill
        # 96.. ; the step loop overwrites rows 96..H1/HM with activations)
        nc.vector.tensor_copy(h1s[96:H1 + 1, :], ones_col[96:H1 + 1, :].broadcast_to([5, MCC]))
        nc.vector.tensor_copy(h2s[96:H1 + 1, :], ones_col[96:H1 + 1, :].broadcast_to([5, MCC]))
        nc.vector.tensor_copy(hm[96:HM + 2, :], ones_col[96:HM + 2, :].broadcast_to([26, MCC]))

        pd = stat.tile([P, G], F32)
        vd = stat.tile([P, G], F32)
        cv = stat.tile([P, G, CW], F32)
        cvfwd = stat.tile([P, G, CW + 1], BF16)   # col Lw = vv (relative cols)
        outacc = stat.tile([1, 2 * NM * NS], F32)
        nc.gpsimd.memset(outacc[:], 0.0)

        sd_tiles = [sd_a, sd_b]

        for rep in range(repeat):
          # per-repeat state init
          nc.vector.tensor_copy(sv_s, initv[:, 0:1].broadcast_to([P, G]))
          nc.vector.tensor_copy(sv_v, initv[:, 1:2].broadcast_to([P, G]))
          nc.vector.tensor_copy(sd_a[:], initv[:, 2:3].broadcast_to([P, G]))
          nc.gpsimd.memset(cv[:], 0.0)
          pending_cv = None                     # deferred cv += cvfwd*dS emitter

          for t, st in enumerate(steps):
            t0, h, sqh = st["t0"], st["h"], st["sqh"]
            rate = st["rate"]
            idx = st["idx"]
            Lw = CW - idx * NS                # live cv width
            spw = Lw + 2                      # live + vv + vd fold cols
            sd_old = sd_tiles[t % 2]
            sd_new = sd_tiles[(t + 1) % 2]

            _scope(nc, "x3")
            # ---- x3: bf16 cast + 16 pair-transposes into [2,2048] psum,
            #      evac'd progressively as four [2,512] copies ----
            # per-wave SVb cast so wave-w transposes gate only on wave-w state
            nc.vector.tensor_copy(SVb[:, 0:16], SV[:, 0:16])
            nc.vector.tensor_copy(SVb[:, 16:32], SV[:, 16:32])
            x3p = ps_x3.tile([2, MCC], BF16, tag="x3p")
            for g in range(G):
                nc.tensor.transpose(x3p[0:2, g * P:(g + 1) * P],
                                    SVb[:, 2 * g:2 * g + 2], identB[:])
            nc.vector.tensor_copy(x3[0:2, 0:1024], x3p[0:2, 0:1024])
            nc.scalar.copy(x3[0:2, 1024:2048], x3p[0:2, 1024:2048])

            # deferred cv-update of the previous step: its deps are long done,
            # so it fills engine idle slots during this step's L1/L2 phase
            if pending_cv is not None:
                pending_cv()
                pending_cv = None

            _scope(nc, "L1")
            # ---- L1 matmuls: sv (100 wide) and merged vh+vdvv (120 wide) ----
            # evacs quarter-width so consumers gate on 512-col granularity
            w1sv_t = w1sv_tab[:, t * H1:(t + 1) * H1]
            w1m_t = w1m_tab[:, t * HM:(t + 1) * HM]
            l1sv = []
            for half in range(2):
                ps = ps_w.tile([P, 8, P], PSD, tag="w")
                for q2 in range(2):
                    nc.tensor.matmul(ps[0:H1].rearrange("p a b -> p (a b)")[:, q2 * 512:(q2 + 1) * 512],
                                     w1sv_t, x3[:, half * 1024 + q2 * 512: half * 1024 + (q2 + 1) * 512])
                l1sv.append(ps)
            nc.scalar.activation(h1s[0:H1, 0:1024], l1sv[0][0:H1].rearrange("p a b -> p (a b)"),
                                 AF.Relu, bias=0.0, scale=1.0)
            nc.vector.tensor_scalar(h1s[0:H1, 1024:2048], l1sv[1][0:H1].rearrange("p a b -> p (a b)"),
                                    0.0, None, OP.max)
            l1m = []
            for half in range(2):
                ps = ps_w.tile([P, 8, P], PSD, tag="w")
                for q2 in range(2):
                    nc.tensor.matmul(ps[0:HM].rearrange("p a b -> p (a b)")[:, q2 * 512:(q2 + 1) * 512],
                                     w1m_t, x3[:, half * 1024 + q2 * 512: half * 1024 + (q2 + 1) * 512])
                l1m.append(ps)
            nc.scalar.activation(hm[0:HM, 0:1024], l1m[0][0:HM].rearrange("p a b -> p (a b)"),
                                 AF.Relu, bias=0.0, scale=1.0)
            nc.vector.tensor_scalar(hm[0:HM, 1024:2048], l1m[1][0:HM].rearrange("p a b -> p (a b)"),
                                    0.0, None, OP.max)

            _scope(nc, "L2")
            # ---- sv L2 ----
            l2 = []
            for half in range(2):
                ps = ps_w.tile([P, 8, P], PSD, tag="w")
                for q in range(2):
                    nc.tensor.matmul(ps[0:H1].rearrange("p a b -> p (a b)")[:, q * 512:(q + 1) * 512],
                                     w2aug[:], h1s[:, half * 1024 + q * 512: half * 1024 + (q + 1) * 512])
                l2.append(ps)
            nc.scalar.activation(h2s[0:H1, 0:1024], l2[0][0:H1].rearrange("p a b -> p (a b)"),
                                 AF.Relu, bias=0.0, scale=1.0)
            nc.vector.tensor_scalar(h2s[0:H1, 1024:2048], l2[1][0:H1].rearrange("p a b -> p (a b)"),
                                    0.0, None, OP.max)

            _scope(nc, "fold")
            # ---- merged fold: live cv cols + vv + vd ----
            # softplus(x) = (SQA*x + SQB)^2 + SPK exactly (= ln2 + x/2 + x^2/8);
            # cvfwd holds only the squared part -- the SPK*dS cv contribution
            # telescopes to SPK*(sd-sd0), applied at maturity events.
            SQA = float(1.0 / np.sqrt(8.0))
            SPK = float(np.log(2.0) - 0.5)
            w2m_live = w2m[:, idx * NS:CW + 2]
            for half in range(2):
                hs = slice(half * 8, (half + 1) * 8)
                ps = ps_w.tile([P, 8, P], PSD, tag="w")
                for gl in range(8):
                    g = half * 8 + gl
                    nc.tensor.matmul(ps[:, gl, 0:spw], hm[:, g * P:(g + 1) * P], w2m_live)
                nc.scalar.activation(cvfwd[:, hs, 0:Lw + 1], ps[:, :, 0:Lw + 1],
                                     AF.Square, bias=sqb[:, 0:1], scale=SQA)
                nc.vector.tensor_copy(vd[:, hs], ps[:, :, Lw + 1])
            vv2_ap = cvfwd[:, :, Lw]          # squared part; true vv = vv2 + SPK

            _scope(nc, "pdfold")
            # ---- sv L3 fold -> pd = (SQA*x+SQB)^2 + SPK ----
            pdf = ps_pdf.tile([P, 2 * G], PSD, tag="pdf")
            for g in range(G):
                nc.tensor.matmul(pdf[:, 2 * g:2 * g + 2], h2s[:, g * P:(g + 1) * P], w3aug[:])
            x2k = work.tile([P, G], F32, tag="x2k")
            vv = work.tile([P, G], F32, tag="vv")
            vtmp = work.tile([P, G], F32, tag="vtmp")
            vvdB = work.tile([P, G], F32, tag="vvdB")
            pd2 = work.tile([P, G], F32, tag="pd2")
            drift = work.tile([P, G], F32, tag="drift")
            absd = work.tile([P, G], F32, tag="absd")
            w1t = work.tile([P, G], F32, tag="w1t")
            term1 = work.tile([P, G], F32, tag="term1")
            a2 = work.tile([P, G], F32, tag="a2")
            w2t = work.tile([P, G], F32, tag="w2t")
            pdz = work.tile([P, G], F32, tag="pdz")
            term2 = work.tile([P, G], F32, tag="term2")
            u = work.tile([P, G], F32, tag="u")
            equ = work.tile([P, G], F32, tag="equ")
            dS = work.tile([P, G], F32, tag="dS")

            # ---- per-wave V / slog / exp chains (waves = chunk halves;
            #      independent dep chains so the scheduler can overlap
            #      wave-1's tail with wave-0's next-step head) ----
            for w in range(2):
                cs = slice(w * 8, (w + 1) * 8)
                sv_s_w = SV[:, 16 * w:16 * (w + 1):2]
                sv_v_w = SV[:, 16 * w + 1:16 * (w + 1):2]
                dB_t = dbt[:, t, cs]
                z_t = zt[:, t, cs]

                _scope(nc, "pdfold")
                nc.scalar.activation(x2k[:, cs], pdf[:, 16 * w:16 * (w + 1):2],
                                     AF.Square, bias=sqb[:, 0:1], scale=SQA)
                nc.vector.tensor_scalar(pd[:, cs], x2k[:, cs], SPK, None, OP.add)

                _scope(nc, "V")
                # V += vd*h + (vv2+SPK)*dB
                nc.vector.tensor_scalar(vv[:, cs], cvfwd[:, cs, Lw], SPK, None, OP.add)
                nc.vector.scalar_tensor_tensor(vtmp[:, cs], vd[:, cs], float(h), sv_v_w,
                                               OP.mult, OP.add)
                nc.vector.tensor_tensor(vvdB[:, cs], vv[:, cs], dB_t, OP.mult)
                nc.vector.tensor_tensor(sv_v_w, vtmp[:, cs], vvdB[:, cs], OP.add)

                _scope(nc, "slog")
                # u = drift*h*(1-|drift|*sqh) + pd*z*sqh*(1 - pd*sqh + (pd*sqh)^2)
                # (1/(1+y) ~ 1-y(+y^2); y<0.05 so err < 1e-5 relative)
                nc.vector.tensor_tensor(pd2[:, cs], pd[:, cs], pd[:, cs], OP.mult)
                nc.vector.tensor_scalar(drift[:, cs], pd2[:, cs], -0.5, float(rate),
                                        OP.mult, OP.add)
                nc.scalar.activation(absd[:, cs], drift[:, cs], AF.Abs, bias=0.0, scale=1.0)
                nc.vector.tensor_scalar(w1t[:, cs], absd[:, cs], float(-h * sqh), float(h),
                                        OP.mult, OP.add)
                nc.vector.tensor_tensor(term1[:, cs], drift[:, cs], w1t[:, cs], OP.mult)
                nc.vector.tensor_scalar(a2[:, cs], pd[:, cs], float(-sqh), 1.0, OP.mult, OP.add)
                nc.vector.scalar_tensor_tensor(w2t[:, cs], pd2[:, cs], float(sqh * sqh),
                                               a2[:, cs], OP.mult, OP.add)
                nc.vector.tensor_tensor(pdz[:, cs], pd[:, cs], z_t, OP.mult)
                nc.vector.scalar_tensor_tensor(term2[:, cs], pdz[:, cs], float(sqh),
                                               w2t[:, cs], OP.mult, OP.mult)
                nc.vector.tensor_tensor(u[:, cs], term1[:, cs], term2[:, cs], OP.add)
                nc.vector.tensor_tensor(sv_s_w, sv_s_w, u[:, cs], OP.add)

                _scope(nc, "exp")
                # sd_new = sd_old * exp(u - r*h)
                nc.scalar.activation(equ[:, cs], u[:, cs], AF.Exp, bias=rhb[:, t:t + 1],
                                     scale=1.0)
                eng = nc.vector if w == 0 else nc.gpsimd
                eng.tensor_tensor(sd_new[:, cs], sd_old[:, cs], equ[:, cs], OP.mult)
                eng.tensor_tensor(dS[:, cs], sd_new[:, cs], sd_old[:, cs], OP.subtract)

            # ---- cv += cvfwd * dS (live cols; chunks split DVE/Pool) ----
            # Emitted deferred (top of next step) unless this step has an
            # event, so the next step's x3/L1 head work leads the queues.
            def emit_cv(dS=dS, idx=idx, Lw=Lw, tno=t):
                _scope(nc, "cv")
                cvds = work.tile([P, G, CW], F32, tag="cvds")
                h0 = slice(0, 8)
                h1_ = slice(8, 16)
                dS_b0 = dS[:, h0].unsqueeze(-1).broadcast_to([P, 8, Lw])
                dS_b1 = dS[:, h1_].unsqueeze(-1).broadcast_to([P, 8, Lw])
                nc.vector.tensor_tensor(cvds[:, h0, 0:Lw], cvfwd[:, h0, 0:Lw], dS_b0, OP.mult)
                nc.gpsimd.tensor_tensor(cvds[:, h1_, 0:Lw], cvfwd[:, h1_, 0:Lw], dS_b1, OP.mult)
                nc.vector.tensor_tensor(cv[:, h0, idx * NS:CW], cv[:, h0, idx * NS:CW],
                                        cvds[:, h0, 0:Lw], OP.add)
                nc.gpsimd.tensor_tensor(cv[:, h1_, idx * NS:CW], cv[:, h1_, idx * NS:CW],
                                        cvds[:, h1_, 0:Lw], OP.add)

            if st["event"] is not None:
                emit_cv()
            else:
                pending_cv = emit_cv

            _scope(nc, "event")
            # ---- maturity event ----
            if st["event"] is not None:
                ev, kslots = st["event"]
                sd0v = st["sd0"]
                pay = work.tile([P, G, NS], F32, tag="pay")
                sd_bc = sd_new[:].unsqueeze(-1).broadcast_to([P, G, NS])
                kd_bc = krep[:, ev * NS:(ev + 1) * NS].unsqueeze(1).broadcast_to([P, G, NS])
                nc.vector.tensor_tensor(pay[:], sd_bc, kd_bc, OP.subtract)
                nc.vector.tensor_scalar(pay[:], pay[:], 0.0, None, OP.max)
                # corr = SPK*(sd_new - sd0): the deferred softplus constant
                corr = work.tile([P, G], F32, tag="corr")
                nc.vector.tensor_scalar(corr[:], sd_new[:], SPK, float(-SPK * sd0v),
                                        OP.mult, OP.add)
                price = work.tile([P, G, NS], F32, tag="price")
                nc.vector.tensor_tensor(price[:], pay[:],
                                        corr[:].unsqueeze(-1).broadcast_to([P, G, NS]), OP.subtract)
                nc.vector.tensor_tensor(price[:], price[:],
                                        cv[:, :, idx * NS:(idx + 1) * NS], OP.subtract)
                price2 = work.tile([P, G, NS], F32, tag="price2")
                nc.vector.tensor_tensor(price2[:], price[:], price[:], OP.mult)
                red = work.tile([P, 2 * NS], F32, tag="red")
                nc.vector.tensor_reduce(red[:, 0:NS], price[:].transpose([0, 2, 1]),
                                        mybir.AxisListType.X, OP.add)
                nc.vector.tensor_reduce(red[:, NS:2 * NS], price2[:].transpose([0, 2, 1]),
                                        mybir.AxisListType.X, OP.add)
                pred = ps_pdf.tile([1, 2 * NS], F32, tag="pred")
                nc.tensor.matmul(pred[:], ones_col[:], red[:])
                for k in kslots:
                    nc.scalar.copy(outacc[0:1, k * NS:(k + 1) * NS], pred[0:1, 0:NS])
                    nc.scalar.copy(outacc[0:1, NM * NS + k * NS:NM * NS + (k + 1) * NS],
                                   pred[0:1, NS:2 * NS])

          if pending_cv is not None:
              pending_cv()
              pending_cv = None

        _scope_end(nc)
        nc.sync.dma_start(out=out_d[:].unsqueeze(0), in_=outacc[:])

    nc.compile()
    return nc


def _prep(inputs):
    """Host-side preprocessing -> (steps, arrays-for-in_maps, shards, written, T)."""
    import ml_dtypes
    bf = ml_dtypes.bfloat16
    f = lambda k: np.asarray(inputs[k], dtype=np.float32)
    S0 = float(f("S0")); rate = float(f("rate"))
    z = f("z"); zz = f("zz")
    timegrid = f("timegrid"); strikes = f("strikes")
    v0 = float(f("v0")[0]); rho = float(f("rho")[0])
    mats = np.asarray(inputs["maturities"]).astype(np.int64)

    rho_t = float(np.tanh(np.float32(rho)))
    c_t = float(np.sqrt(np.float32(1.0) - np.float32(rho_t) ** 2))
    V0 = float(1.0 / (1.0 + np.exp(-np.float32(v0))) * 0.5)
    slog0 = float(np.log(np.float32(S0)))

    days = np.round(timegrid * 365.0).astype(np.int64)
    le = days[1:, None] <= mats[None, :]
    idx_net = np.argmax(le, axis=1)
    is_mat = np.any(days[1:, None] == mats[None, :], axis=1)
    if not is_mat.any():
        return None

    T = int(np.max(np.nonzero(is_mat)[0])) + 1
    steps = []
    krep_list = []
    ev = 0
    for t in range(T):
        t0 = float(timegrid[t]); t1 = float(timegrid[t + 1])
        h = float(np.float32(t1) - np.float32(t0))
        sqh = float(np.sqrt(np.float32(h)))
        event = None
        if is_mat[t]:
            k = int(idx_net[t])
            event = (ev, [k])
            krep_list.append(np.exp(-rate * t1).astype(np.float32) * strikes)
            ev += 1
        steps.append(dict(
            t0=t0, h=h, sqh=sqh, rho_s=rho_t * sqh, c_s=c_t * sqh, rate=rate,
            sd0=float(np.exp(np.float32(slog0) - np.float32(rate) * timegrid[0])),
            idx=int(idx_net[t]), event=event,
        ))

    # weight repacks
    sv_W1 = f("sv_W1"); sv_b1 = f("sv_b1"); sv_W2 = f("sv_W2"); sv_b2 = f("sv_b2")
    sv_W3 = f("sv_W3"); sv_b3 = f("sv_b3")
    vh_W1 = f("vh_W1"); vh_b1 = f("vh_b1"); vh_W2 = f("vh_W2"); vh_b2 = f("vh_b2")
    vd_W1 = f("vd_W1"); vd_b1 = f("vd_b1"); vd_W2 = f("vd_W2"); vd_b2 = f("vd_b2")
    vv_W1 = f("vv_W1"); vv_b1 = f("vv_b1"); vv_W2 = f("vv_W2"); vv_b2 = f("vv_b2")
    t0s = timegrid[:T].astype(np.float32)

    arrs = {}
    # w1sv_tab rows (slog, v, bias(t))
    w1sv_3 = np.zeros((3, T * H1), np.float32)
    for t in range(T):
        w1sv_3[0, t * H1:(t + 1) * H1] = sv_W1[1]
        w1sv_3[1, t * H1:(t + 1) * H1] = sv_W1[2]
        w1sv_3[2, t * H1:(t + 1) * H1] = sv_b1 + sv_W1[0] * t0s[t]
    arrs["w1sv_tab"] = w1sv_3
    # w1m_tab: cols 0-79 vh (slog row + t-bias), cols 80-119 vdvv (v row + bias)
    w1m_3 = np.zeros((3, T * HM), np.float32)
    vh_w1_t = vh_W1[:, 0, :].reshape(NM * VH)   # t0 weight
    vh_w1_s = vh_W1[:, 1, :].reshape(NM * VH)   # slog weight
    vh_b1f = vh_b1.reshape(NM * VH)
    for t in range(T):
        c0 = t * HM
        w1m_3[0, c0:c0 + NM * VH] = vh_w1_s
        w1m_3[2, c0:c0 + NM * VH] = vh_b1f + vh_w1_t * t0s[t]
        w1m_3[1, c0 + NM * VH:c0 + NM * VH + DV] = vd_W1[0]
        w1m_3[2, c0 + NM * VH:c0 + NM * VH + DV] = vd_b1
        w1m_3[1, c0 + NM * VH + DV:c0 + HM] = vv_W1[0]
        w1m_3[2, c0 + NM * VH + DV:c0 + HM] = vv_b1
    arrs["w1m_tab"] = w1m_3
    arrs["w2aug"] = np.concatenate([sv_W2, sv_b2[None, :]], 0)
    arrs["w3aug"] = np.concatenate(
        [np.concatenate([sv_W3, sv_b3[None, :]], 0), np.zeros((H1 + 1, 1), np.float32)], 1)
    # w2m [122, 86]: vh block-diag + vv/vd columns; rows 120/121 are the two ones rows
    w2m = np.zeros((HM + 2, CW + 2), np.float32)
    for k in range(NM):
        w2m[k * VH:(k + 1) * VH, k * NS:(k + 1) * NS] = vh_W2[k]
        w2m[HM, k * NS:(k + 1) * NS] = vh_b2[k]
    w2m[NM * VH + DV:HM, CW] = vv_W2[:, 0]
    w2m[NM * VH:NM * VH + DV, CW + 1] = vd_W2[:, 0]
    w2m[HM + 1, CW] = vv_b2[0]
    w2m[HM + 1, CW + 1] = vd_b2[0]
    arrs["w2m"] = w2m
    for k in ("w1sv_tab", "w1m_tab", "w2aug", "w3aug", "w2m"):
        arrs[k] = np.ascontiguousarray(arrs[k]).astype(bf)

    if krep_list:
        arrs["krep"] = np.concatenate(krep_list)[None, :].astype(np.float32)
    else:
        arrs["krep"] = np.zeros((1, NS), np.float32)
    sd0 = float(np.exp(np.float32(slog0) - np.float32(rate) * timegrid[0]))
    arrs["initvals"] = np.array([[slog0, V0, sd0, 1.0]], np.float32)
    arrs["rhb"] = np.array([[-s["rate"] * s["h"] for s in steps]], np.float32)

    # z / dB shards: [MCC, T] slice -> [G, P, T] -> [P, T, G]
    rho_s = np.array([s["rho_s"] for s in steps], np.float32)  # [T]
    c_s = np.array([s["c_s"] for s in steps], np.float32)
    zshards, dbshards = [], []
    for c in range(N_CORES):
        zc = z[c * MCC:(c + 1) * MCC, :T]
        zzc = zz[c * MCC:(c + 1) * MCC, :T]
        dbc = rho_s[None, :] * zc + c_s[None, :] * zzc
        for src, lst in ((zc, zshards), (dbc, dbshards)):
            s = src.reshape(G, P, T).transpose(1, 2, 0)
            lst.append(np.ascontiguousarray(s, dtype=np.float32))

    written = sorted({k for s in steps if s["event"] for k in s["event"][1]})
    return steps, arrs, zshards, dbshards, written, T


_CACHE = {}


def kernel(**inputs) -> np.ndarray:
    prep = _prep(inputs)
    if prep is None:
        return np.zeros((2, NM, NS), np.float32)
    steps, arrs, zshards, dbshards, written, T = prep

    key = (T,) + tuple(
        (s["t0"], s["h"], s["rho_s"], s["c_s"], s["rate"], s["idx"],
         None if s["event"] is None else (s["event"][0], tuple(s["event"][1])))
        for s in steps)
    nc = _CACHE.get(key)
    if nc is None:
        nc = build_program(steps)
        _CACHE[key] = nc

    in_maps = []
    for c in range(N_CORES):
        m = dict(arrs)
        m["z_land"] = zshards[c]
        m["db_land"] = dbshards[c]
        in_maps.append(m)

    res = run_bass_kernel_spmd(nc, in_maps, list(range(N_CORES)))
    sums = np.zeros(2 * NM * NS, np.float64)
    for c in range(N_CORES):
        sums += res.results[c]["out"].astype(np.float64)
    s1 = sums[:NM * NS].reshape(NM, NS)
    s2 = sums[NM * NS:].reshape(NM, NS)
    pv = np.zeros((NM, NS), np.float64)
    pvar = np.zeros((NM, NS), np.float64)
    for k in written:
        pv[k] = s1[k] / MC
        pvar[k] = (s2[k] - MC * pv[k] ** 2) / (MC - 1)
    return np.stack([pv, pvar]).astype(np.float32)
